# revision 9
# baseline (speedup 1.0000x reference)
"""CBOW negative-sampling loss kernel for Trainium2 (8 NeuronCores, Bass/Tile).

Sharding: data-parallel over the batch dim (16384 -> 8 x 2048 items), embedding
tables replicated per core.

The fast HBM gather primitive (gpsimd dma_gather) takes int16 row indices, so
the 100000-row tables are addressed through four 32768-row windows.  Per core:

- ctx stream: the 2048*10 context rows, sorted by window, gathered compacted
  with a handful of dma_gather calls (f32), cast to bf16 on the scalar engine,
  then dma_scatter_add'ed into unique canonical (item, c) SBUF slots
  (parity-split pair of accumulators; slot collisions impossible, pads go to a
  dustbin slot).  DVE tree-adds reduce the 10 context slots -> ctx_sum per
  item; +ctx_sum and -ctx_sum are staged to DRAM (bf16).
- out stream: the 2048*11 target+negative rows, sorted by window, gathered
  compacted (f32 -> bf16 cast); each row's +-ctx_sum partner is fetched with a
  second dma_gather from the staging table (index = item, or item+ITEMS for
  target rows so the sign of the score is baked in; pads hit a zero row).
  Elementwise product + innermost reduce gives the raw scores in stream order.
- softplus(0.1*score) = Ln(1+Exp(0.1*score)) on the scalar engine with a
  per-partition accumulator.  Stream pads contribute exactly ln(2) each; the
  host subtracts the (static) pad count and divides by B.
"""
import math
import sys

if '/opt/trn_rl_repo' not in sys.path:
    sys.path.insert(0, '/opt/trn_rl_repo')

import numpy as np

P = 128          # partitions
D = 128          # embedding dim
CTX = 10         # context window
NOUT = 11        # 1 target + 10 negatives
V = 100000       # vocab rows
WIN = 32768      # int16 gather window
NCORES = 8
CHUNK = 1024     # max rows per gather/scatter call (Q7 idx scratch bound)

_PROGRAM_CACHE = {}
LN2 = float(np.log(np.float32(2.0)))


def _round_up(x, m):
    return ((x + m - 1) // m) * m


def _window_caps(n, v=V, win=WIN, sigmas=8.0):
    """Static per-window stream capacities: mean + sigmas*sigma, 128-aligned."""
    nw = (v + win - 1) // win
    caps = []
    for w in range(nw):
        p = (min(win * (w + 1), v) - win * w) / v
        mean = n * p
        sig = math.sqrt(n * p * (1 - p))
        caps.append(_round_up(int(mean + sigmas * sig) + 1, P))
    return caps


def _chunks_of(cap):
    out = []
    while cap > 0:
        c = min(cap, CHUNK)
        out.append(c)
        cap -= c
    return out


class _Plan:
    """Static program layout for a per-core batch of T*128 items."""

    def __init__(self, T):
        self.T = T
        self.items = T * P
        n_ctx = self.items * CTX
        n_out = self.items * NOUT
        self.ctx_caps = _window_caps(n_ctx)
        self.out_caps = _window_caps(n_out)
        self.nw = len(self.ctx_caps)
        # (window, nrows) call list
        self.ctx_calls = [(w, c) for w in range(self.nw)
                          for c in _chunks_of(self.ctx_caps[w])]
        self.out_calls = [(w, c) for w in range(self.nw)
                          for c in _chunks_of(self.out_caps[w])]
        self.ctx_total = sum(self.ctx_caps)
        self.out_total = sum(self.out_caps)
        self.out_pads = self.out_total - n_out     # each contributes ln2
        self.score_cols = self.out_total // P
        # canonical ctx slot space: t = CTX*tile + c in [0, CTX*T); dustbin t
        self.t_dust_ctx = CTX * T
        self.ctx_groups = (self.t_dust_ctx + 2) // 2   # per parity buffer
        self.s_dust_ctx = (P - 1) + P * self.t_dust_ctx
        # staging rows: [0,items) = +ctx_sum, [items, 2*items) = -ctx_sum,
        # [2*items, 2*items+P) = zeros (pad target)
        self.z_row = 2 * self.items
        self.staging_rows = 2 * self.items + P


def _host_prep_core(plan, ctx_ids, tgt_ids, neg_ids):
    """Build the per-core index tensors.

    ctx_ids [items, CTX], tgt_ids [items], neg_ids [items, CTX] (int64/32).
    Returns dict of np arrays + overflow check.
    """
    items = plan.items
    T = plan.T

    def wrap(vals):
        # dma_gather idx layout: idx[i] read from [i%16, i//16]; replicate to
        # 128 partitions (the ucode reads a queue-dependent 16-partition band).
        n = len(vals)
        assert n % 16 == 0
        arr = np.asarray(vals, np.int16).reshape(n // 16, 16).T  # [16, n/16]
        return np.tile(arr, (8, 1))                              # [128, n/16]

    item_idx = np.arange(items)
    tile = item_idx // P
    part = item_idx % P

    # --- ctx stream ---
    ids = np.asarray(ctx_ids, np.int64)
    w_of = np.minimum(ids // WIN, plan.nw - 1)
    rel = ids - w_of * WIN
    slot = part[:, None] + P * (CTX * tile[:, None] + np.arange(CTX)[None, :])
    ctx_g, ctx_s = [], []
    for w in range(plan.nw):
        sel = (w_of == w)
        g = rel[sel]
        s = slot[sel]
        cap = plan.ctx_caps[w]
        if len(g) > cap:
            raise RuntimeError(
                f"ctx window {w} overflow: {len(g)} > cap {cap}")
        pad = cap - len(g)
        ctx_g.append(np.concatenate([g, np.zeros(pad, np.int64)]))
        ctx_s.append(np.concatenate([s, np.full(pad, plan.s_dust_ctx)]))
    ctx_g = np.concatenate(ctx_g)
    ctx_s = np.concatenate(ctx_s)

    # --- out stream ---
    oids = np.concatenate(
        [np.asarray(tgt_ids, np.int64)[:, None],
         np.asarray(neg_ids, np.int64)], axis=1)          # [items, NOUT]
    w_of = np.minimum(oids // WIN, plan.nw - 1)
    rel = oids - w_of * WIN
    # pairing idx into staging: +ctx for negatives, -ctx (row items+item) for
    # the target column j=0
    pairs = item_idx[:, None] + plan.items * (np.arange(NOUT)[None, :] == 0)
    out_g, out_p = [], []
    for w in range(plan.nw):
        sel = (w_of == w)
        g = rel[sel]
        pr = pairs[sel]
        cap = plan.out_caps[w]
        if len(g) > cap:
            raise RuntimeError(
                f"out window {w} overflow: {len(g)} > cap {cap}")
        pad = cap - len(g)
        out_g.append(np.concatenate([g, np.zeros(pad, np.int64)]))
        out_p.append(np.concatenate([pr, np.full(pad, plan.z_row)]))
    out_g = np.concatenate(out_g)
    out_p = np.concatenate(out_p)

    return {
        "ctx_gidx": wrap(ctx_g),
        "ctx_sidx": wrap(ctx_s),
        "out_gidx": wrap(out_g),
        "out_pidx": wrap(out_p),
    }


def _build_program(plan):
    from contextlib import ExitStack

    import concourse.bacc as bacc
    import concourse.bass as bass
    import concourse.mybir as mybir
    import concourse.tile as tile
    from concourse.library_config import mlp as mlp_lib

    T = plan.T
    f32 = mybir.dt.float32
    bf16 = mybir.dt.bfloat16
    i16 = mybir.dt.int16
    AL = mybir.AluOpType
    AF = mybir.ActivationFunctionType

    nc = bacc.Bacc("TRN2", num_swdge_queues=1)

    w_embed = nc.dram_tensor("w_embed", (V, D), f32, kind="ExternalInput")
    w_out = nc.dram_tensor("w_out", (V, D), f32, kind="ExternalInput")
    ctx_gidx = nc.dram_tensor("ctx_gidx", (P, plan.ctx_total // 16), i16,
                              kind="ExternalInput")
    ctx_sidx = nc.dram_tensor("ctx_sidx", (P, plan.ctx_total // 16), i16,
                              kind="ExternalInput")
    out_gidx = nc.dram_tensor("out_gidx", (P, plan.out_total // 16), i16,
                              kind="ExternalInput")
    out_pidx = nc.dram_tensor("out_pidx", (P, plan.out_total // 16), i16,
                              kind="ExternalInput")
    out = nc.dram_tensor("out", (P, 1), f32, kind="ExternalOutput")
    staging = nc.dram_tensor("staging", (plan.staging_rows, D), bf16,
                             kind="Internal")

    win_starts = [w * WIN for w in range(plan.nw)]
    win_ends = [min((w + 1) * WIN, V) for w in range(plan.nw)]

    with tile.TileContext(nc) as tc, ExitStack() as ctx:
        cpool = ctx.enter_context(tc.tile_pool(name="const", bufs=1))
        gpool = ctx.enter_context(tc.tile_pool(name="work", bufs=3))
        tpool = ctx.enter_context(tc.tile_pool(name="tree", bufs=1))

        nc.gpsimd.load_library(mlp_lib)

        # index tiles
        cg = cpool.tile([P, plan.ctx_total // 16], i16)
        cs = cpool.tile([P, plan.ctx_total // 16], i16)
        og = cpool.tile([P, plan.out_total // 16], i16)
        op = cpool.tile([P, plan.out_total // 16], i16)
        nc.sync.dma_start(out=cg[:], in_=ctx_gidx[:][:, :])
        nc.sync.dma_start(out=cs[:], in_=ctx_sidx[:][:, :])
        nc.sync.dma_start(out=og[:], in_=out_gidx[:][:, :])
        nc.sync.dma_start(out=op[:], in_=out_pidx[:][:, :])

        # canonical ctx accumulators (parity split) + zero them
        acc_e = cpool.tile([P, plan.ctx_groups, D], bf16)
        acc_o = cpool.tile([P, plan.ctx_groups, D], bf16)
        nc.vector.memset(acc_e[:], 0.0)
        nc.vector.memset(acc_o[:], 0.0)

        score_buf = cpool.tile([P, plan.score_cols], f32)

        with nc.allow_low_precision("bf16 pipeline validated vs f32 ref"):
            # ---- ctx phase ----
            off = 0
            for ci, (w, n) in enumerate(plan.ctx_calls):
                g = n // 16
                gr = n // P
                raw = gpool.tile([P, CHUNK // P, D], f32, tag="raw")
                nc.gpsimd.dma_gather(
                    out_ap=raw[:, :gr, :],
                    in_ap=w_embed[win_starts[w]:win_ends[w], :],
                    idxs_ap=cg[:, off:off + g],
                    num_idxs=n,
                    num_idxs_reg=n,
                    elem_size=D,
                    queue_num=0,
                )
                b16 = gpool.tile([P, CHUNK // P, D], bf16, tag="b16")
                nc.scalar.activation(
                    out=b16[:, :gr, :], in_=raw[:, :gr, :], func=AF.Copy)
                nc.gpsimd.dma_scatter_add(
                    out_ap=acc_e[:],
                    in_ap=b16[:, :gr, :],
                    idxs_ap=cs[:, off:off + g],
                    num_idxs=n,
                    num_idxs_reg=n,
                    elem_size=D,
                    queue_num=0,
                    sbuf_tokens_per_rank=P,
                    parity_reg=0,
                    out_ap_other=acc_o[:],
                )
                off += g

            # ---- ctx reduce: t = CTX*tile + c ; even c in acc_e at
            # g = (CTX//2)*tile + c//2, odd c in acc_o likewise ----
            H = CTX // 2  # 5
            ve = acc_e[:, :T * H, :].rearrange("p (t h) d -> p t h d", h=H)
            vo = acc_o[:, :T * H, :].rearrange("p (t h) d -> p t h d", h=H)
            t2 = tpool.tile([P, T, 2, D], bf16, tag="t2")
            nc.vector.tensor_tensor(
                out=t2[:], in0=ve[:, :, 0:2, :], in1=ve[:, :, 2:4, :], op=AL.add)
            u2 = tpool.tile([P, T, 2, D], bf16, tag="u2")
            nc.vector.tensor_tensor(
                out=u2[:], in0=vo[:, :, 0:2, :], in1=vo[:, :, 2:4, :], op=AL.add)
            t1 = tpool.tile([P, T, 1, D], bf16, tag="t1")
            nc.vector.tensor_tensor(
                out=t1[:], in0=t2[:, :, 0:1, :], in1=t2[:, :, 1:2, :], op=AL.add)
            u1 = tpool.tile([P, T, 1, D], bf16, tag="u1")
            nc.vector.tensor_tensor(
                out=u1[:], in0=u2[:, :, 0:1, :], in1=u2[:, :, 1:2, :], op=AL.add)
            t0 = tpool.tile([P, T, 1, D], bf16, tag="t0")
            nc.vector.tensor_tensor(
                out=t0[:], in0=t1[:], in1=ve[:, :, 4:5, :], op=AL.add)
            u0 = tpool.tile([P, T, 1, D], bf16, tag="u0")
            nc.vector.tensor_tensor(
                out=u0[:], in0=u1[:], in1=vo[:, :, 4:5, :], op=AL.add)
            united = cpool.tile([P, T, D], bf16)
            nc.vector.tensor_tensor(
                out=united[:], in0=t0[:, :, 0, :], in1=u0[:, :, 0, :], op=AL.add)
            negated = cpool.tile([P, T, D], bf16)
            nc.vector.tensor_scalar(
                out=negated[:], in0=united[:], scalar1=-1.0, scalar2=None,
                op0=AL.mult)

            # ---- stage +-ctx_sum and the zero row ----
            pos_view = staging[0:plan.items, :].rearrange(
                "(t p) d -> p t d", p=P)
            neg_view = staging[plan.items:2 * plan.items, :].rearrange(
                "(t p) d -> p t d", p=P)
            nc.sync.dma_start(out=pos_view, in_=united[:])
            nc.sync.dma_start(out=neg_view, in_=negated[:])
            zrow = cpool.tile([P, 1, D], bf16)
            nc.vector.memset(zrow[:], 0.0)
            z_view = staging[plan.z_row:plan.z_row + P, :].rearrange(
                "(a p) d -> p a d", p=P)
            nc.sync.dma_start(out=z_view, in_=zrow[:])

            # ---- out phase ----
            off = 0
            col = 0
            for oi, (w, n) in enumerate(plan.out_calls):
                g = n // 16
                gr = n // P
                raw = gpool.tile([P, CHUNK // P, D], f32, tag="raw")
                nc.gpsimd.dma_gather(
                    out_ap=raw[:, :gr, :],
                    in_ap=w_out[win_starts[w]:win_ends[w], :],
                    idxs_ap=og[:, off:off + g],
                    num_idxs=n,
                    num_idxs_reg=n,
                    elem_size=D,
                    queue_num=0,
                )
                ob = gpool.tile([P, CHUNK // P, D], bf16, tag="b16")
                nc.scalar.activation(
                    out=ob[:, :gr, :], in_=raw[:, :gr, :], func=AF.Copy)
                pair = gpool.tile([P, CHUNK // P, D], bf16, tag="pair")
                nc.gpsimd.dma_gather(
                    out_ap=pair[:, :gr, :],
                    in_ap=staging[:][:, :],
                    idxs_ap=op[:, off:off + g],
                    num_idxs=n,
                    num_idxs_reg=n,
                    elem_size=D,
                    queue_num=0,
                )
                nc.vector.tensor_tensor(
                    out=ob[:, :gr, :], in0=ob[:, :gr, :], in1=pair[:, :gr, :],
                    op=AL.mult)
                nc.vector.tensor_reduce(
                    out=score_buf[:, col:col + gr],
                    in_=ob[:, :gr, :],
                    axis=mybir.AxisListType.X,
                    op=AL.add)
                off += g
                col += gr

            # ---- softplus + accumulate ----
            es = cpool.tile([P, plan.score_cols], f32)
            nc.scalar.activation(
                out=es[:], in_=score_buf[:], func=AF.Exp, scale=1.0 / CTX)
            ls = cpool.tile([P, plan.score_cols], f32)
            acc = cpool.tile([P, 1], f32)
            nc.scalar.activation(
                out=ls[:], in_=es[:], func=AF.Ln, bias=1.0, accum_out=acc[:])
            nc.sync.dma_start(out=out[:][:, :], in_=acc[:])

    if not nc.is_finalized():
        nc.finalize()
    return nc


def _get_program(T):
    if T not in _PROGRAM_CACHE:
        _PROGRAM_CACHE[T] = _build_program(_Plan(T))
    return _PROGRAM_CACHE[T]


def _prep_inputs(W_embed, W_out, context_ids, target_ids, neg_ids):
    B = context_ids.shape[0]
    assert B % (NCORES * P) == 0, B
    T = B // (NCORES * P)
    plan = _Plan(T)

    w_e = np.ascontiguousarray(np.asarray(W_embed, dtype=np.float32))
    w_o = np.ascontiguousarray(np.asarray(W_out, dtype=np.float32))
    ctx = np.asarray(context_ids).reshape(NCORES, plan.items, CTX)
    tgt = np.asarray(target_ids).reshape(NCORES, plan.items)
    neg = np.asarray(neg_ids).reshape(NCORES, plan.items, NOUT - 1)

    in_maps = []
    for c in range(NCORES):
        m = _host_prep_core(plan, ctx[c], tgt[c], neg[c])
        m["w_embed"] = w_e
        m["w_out"] = w_o
        in_maps.append(m)
    return in_maps, B, T, plan


def _run(W_embed, W_out, context_ids, target_ids, neg_ids, **spmd_kwargs):
    from concourse import bass_utils

    in_maps, B, T, plan = _prep_inputs(
        W_embed, W_out, context_ids, target_ids, neg_ids)
    nc = _get_program(T)
    res = bass_utils.run_bass_kernel_spmd(
        nc, in_maps, core_ids=list(range(NCORES)), **spmd_kwargs)
    total = 0.0
    for r in res.results:
        total += float(r["out"].astype(np.float64).sum())
    total -= NCORES * plan.out_pads * LN2
    loss = np.float32(total / B)
    return loss, res


def kernel(W_embed, W_out, context_ids, target_ids, neg_ids):
    loss, _ = _run(W_embed, W_out, context_ids, target_ids, neg_ids)
    return loss


# revision 17
# speedup vs baseline: 23.7133x; 23.7133x over previous
"""CBOW negative-sampling loss kernel for Trainium2 (8 NeuronCores, Bass/Tile).

Sharding: data-parallel over the batch dim (16384 -> 8 x 2048 items), embedding
tables replicated per core.

The fast HBM gather primitive (gpsimd dma_gather) takes int16 row indices, so
the 100000-row tables are addressed through four 32768-row windows.  Per core:

- ctx stream: the 2048*10 context rows, sorted by window, gathered compacted
  with a handful of dma_gather calls (f32), cast to bf16 on the scalar engine,
  then dma_scatter_add'ed into unique canonical (item, c) SBUF slots
  (parity-split pair of accumulators; slot collisions impossible, pads go to a
  dustbin slot).  DVE tree-adds reduce the 10 context slots -> ctx_sum per
  item; +ctx_sum and -ctx_sum are staged to DRAM (bf16).
- out stream: the 2048*11 target+negative rows, sorted by window, gathered
  compacted (f32 -> bf16 cast); each row's +-ctx_sum partner is fetched with a
  second dma_gather from the staging table (index = item, or item+ITEMS for
  target rows so the sign of the score is baked in; pads hit a zero row).
  Elementwise product + innermost reduce gives the raw scores in stream order.
- softplus(0.1*score) = Ln(1+Exp(0.1*score)) on the scalar engine with a
  per-partition accumulator.  Stream pads contribute exactly ln(2) each; the
  host subtracts the (static) pad count and divides by B.
"""
import math
import sys

if '/opt/trn_rl_repo' not in sys.path:
    sys.path.insert(0, '/opt/trn_rl_repo')

import numpy as np

P = 128          # partitions
D = 128          # embedding dim
CTX = 10         # context window
NOUT = 11        # 1 target + 10 negatives
V = 100000       # vocab rows
WIN = 32768      # int16 gather window
NCORES = 8
CHUNK = 1024     # max rows per gather/scatter call (desc-ring bound)
SCHUNK = 1024    # max rows per scatter-add call

_PROGRAM_CACHE = {}
LN2 = float(np.log(np.float32(2.0)))


def _round_up(x, m):
    return ((x + m - 1) // m) * m


def _window_caps(n, v=V, win=WIN, sigmas=8.0):
    """Static per-window stream capacities: mean + sigmas*sigma, 128-aligned."""
    nw = (v + win - 1) // win
    caps = []
    for w in range(nw):
        p = (min(win * (w + 1), v) - win * w) / v
        mean = n * p
        sig = math.sqrt(n * p * (1 - p))
        caps.append(_round_up(int(mean + sigmas * sig) + 1, P))
    return caps


def _chunks_of(cap):
    out = []
    while cap > 0:
        c = min(cap, CHUNK)
        out.append(c)
        cap -= c
    return out


class _Plan:
    """Static program layout for a per-core batch of T*128 items."""

    def __init__(self, T):
        self.T = T
        self.items = T * P
        n_ctx = self.items * CTX
        n_out = self.items * NOUT
        self.ctx_caps = _window_caps(n_ctx)
        self.out_caps = _window_caps(n_out)
        self.nw = len(self.ctx_caps)
        # (window, nrows) call list
        self.ctx_calls = [(w, c) for w in range(self.nw)
                          for c in _chunks_of(self.ctx_caps[w])]
        self.out_calls = [(w, c) for w in range(self.nw)
                          for c in _chunks_of(self.out_caps[w])]
        self.ctx_total = sum(self.ctx_caps)
        self.out_total = sum(self.out_caps)
        self.out_pads = self.out_total - n_out     # each contributes ln2
        self.score_cols = self.out_total // P
        # canonical ctx slot space: t = CTX*tile + c in [0, CTX*T); dustbin t
        self.t_dust_ctx = CTX * T
        self.ctx_groups = (self.t_dust_ctx + 2) // 2   # per parity buffer
        self.s_dust_ctx = (P - 1) + P * self.t_dust_ctx
        # staging rows: [0,items) = +ctx_sum, [items, 2*items) = -ctx_sum,
        # [2*items, 2*items+P) = zeros (pad target)
        self.z_row = 2 * self.items
        self.staging_rows = 2 * self.items + P


def _host_prep_core(plan, ctx_ids, tgt_ids, neg_ids):
    """Build the per-core index tensors.

    ctx_ids [items, CTX], tgt_ids [items], neg_ids [items, CTX] (int64/32).
    Returns dict of np arrays + overflow check.
    """
    items = plan.items
    T = plan.T

    def wrap(vals):
        # dma_gather idx layout: idx[i] read from [i%16, i//16]; replicate to
        # 128 partitions (the ucode reads a queue-dependent 16-partition band).
        n = len(vals)
        assert n % 16 == 0
        arr = np.asarray(vals, np.int16).reshape(n // 16, 16).T  # [16, n/16]
        return np.tile(arr, (8, 1))                              # [128, n/16]

    item_idx = np.arange(items)
    tile = item_idx // P
    part = item_idx % P

    # --- ctx stream ---
    ids = np.asarray(ctx_ids, np.int64)
    w_of = np.minimum(ids // WIN, plan.nw - 1)
    rel = ids - w_of * WIN
    slot = part[:, None] + P * (CTX * tile[:, None] + np.arange(CTX)[None, :])
    ctx_g, ctx_s = [], []
    for w in range(plan.nw):
        sel = (w_of == w)
        g = rel[sel]
        s = slot[sel]
        cap = plan.ctx_caps[w]
        if len(g) > cap:
            raise RuntimeError(
                f"ctx window {w} overflow: {len(g)} > cap {cap}")
        pad = cap - len(g)
        ctx_g.append(np.concatenate([g, np.zeros(pad, np.int64)]))
        ctx_s.append(np.concatenate([s, np.full(pad, plan.s_dust_ctx)]))
    ctx_g = np.concatenate(ctx_g)
    ctx_s = np.concatenate(ctx_s)

    # --- out stream ---
    oids = np.concatenate(
        [np.asarray(tgt_ids, np.int64)[:, None],
         np.asarray(neg_ids, np.int64)], axis=1)          # [items, NOUT]
    w_of = np.minimum(oids // WIN, plan.nw - 1)
    rel = oids - w_of * WIN
    # pairing idx into staging: +ctx for negatives, -ctx (row items+item) for
    # the target column j=0
    pairs = item_idx[:, None] + plan.items * (np.arange(NOUT)[None, :] == 0)
    out_g, out_p = [], []
    for w in range(plan.nw):
        sel = (w_of == w)
        g = rel[sel]
        pr = pairs[sel]
        cap = plan.out_caps[w]
        if len(g) > cap:
            raise RuntimeError(
                f"out window {w} overflow: {len(g)} > cap {cap}")
        pad = cap - len(g)
        out_g.append(np.concatenate([g, np.zeros(pad, np.int64)]))
        out_p.append(np.concatenate([pr, np.full(pad, plan.z_row)]))
    out_g = np.concatenate(out_g)
    out_p = np.concatenate(out_p)

    return {
        "ctx_gidx": wrap(ctx_g),
        "ctx_sidx": wrap(ctx_s),
        "out_gidx": wrap(out_g),
        "out_pidx": wrap(out_p),
    }


def _build_program(plan, repeat=1):
    from contextlib import ExitStack

    import concourse.bacc as bacc
    import concourse.bass as bass
    import concourse.mybir as mybir
    import concourse.tile as tile
    from concourse.library_config import mlp as mlp_lib

    T = plan.T
    f32 = mybir.dt.float32
    bf16 = mybir.dt.bfloat16
    i16 = mybir.dt.int16
    AL = mybir.AluOpType
    AF = mybir.ActivationFunctionType

    nc = bacc.Bacc("TRN2", num_swdge_queues=1)

    w_embed = nc.dram_tensor("w_embed", (V, D), f32, kind="ExternalInput")
    w_out = nc.dram_tensor("w_out", (V, D), f32, kind="ExternalInput")
    ctx_gidx = nc.dram_tensor("ctx_gidx", (P, plan.ctx_total // 16), i16,
                              kind="ExternalInput")
    ctx_sidx = nc.dram_tensor("ctx_sidx", (P, plan.ctx_total // 16), i16,
                              kind="ExternalInput")
    out_gidx = nc.dram_tensor("out_gidx", (P, plan.out_total // 16), i16,
                              kind="ExternalInput")
    out_pidx = nc.dram_tensor("out_pidx", (P, plan.out_total // 16), i16,
                              kind="ExternalInput")
    out = nc.dram_tensor("out", (P, 1), f32, kind="ExternalOutput")
    staging = nc.dram_tensor("staging", (plan.staging_rows, D), bf16,
                             kind="Internal")

    win_starts = [w * WIN for w in range(plan.nw)]
    win_ends = [min((w + 1) * WIN, V) for w in range(plan.nw)]

    with tile.TileContext(nc) as tc, ExitStack() as ctx:
        cpool = ctx.enter_context(tc.tile_pool(name="const", bufs=1))
        gpool = ctx.enter_context(tc.tile_pool(name="work", bufs=8))
        tpool = ctx.enter_context(tc.tile_pool(name="tree", bufs=1))

        nc.gpsimd.load_library(mlp_lib)

        # index tiles
        cg = cpool.tile([P, plan.ctx_total // 16], i16)
        cs = cpool.tile([P, plan.ctx_total // 16], i16)
        og = cpool.tile([P, plan.out_total // 16], i16)
        op = cpool.tile([P, plan.out_total // 16], i16)
        nc.sync.dma_start(out=cg[:], in_=ctx_gidx[:][:, :])
        nc.sync.dma_start(out=cs[:], in_=ctx_sidx[:][:, :])
        nc.sync.dma_start(out=og[:], in_=out_gidx[:][:, :])
        nc.sync.dma_start(out=op[:], in_=out_pidx[:][:, :])

        with nc.allow_low_precision("bf16 pipeline validated vs f32 ref"):
          for _rep in range(repeat):
            # canonical ctx accumulators (parity split) + zero them
            acc_e = cpool.tile([P, plan.ctx_groups, D], bf16, tag="acc_e")
            acc_o = cpool.tile([P, plan.ctx_groups, D], bf16, tag="acc_o")
            nc.vector.memset(acc_e[:], 0.0)
            nc.vector.memset(acc_o[:], 0.0)

            score_buf = cpool.tile([P, plan.score_cols], f32, tag="score_buf")

            # ---- ctx phase ----
            off = 0
            for ci, (w, n) in enumerate(plan.ctx_calls):
                g = n // 16
                gr = n // P
                raw = gpool.tile([P, CHUNK // P, D], f32, tag="raw")
                nc.gpsimd.dma_gather(
                    out_ap=raw[:, :gr, :],
                    in_ap=w_embed[win_starts[w]:win_ends[w], :],
                    idxs_ap=cg[:, off:off + g],
                    num_idxs=n,
                    num_idxs_reg=n,
                    elem_size=D,
                    queue_num=0,
                )
                b16 = gpool.tile([P, CHUNK // P, D], bf16, tag="b16")
                nc.scalar.activation(
                    out=b16[:, :gr, :], in_=raw[:, :gr, :], func=AF.Copy)
                for s0 in range(0, n, SCHUNK):
                    sn = min(SCHUNK, n - s0)
                    nc.gpsimd.dma_scatter_add(
                        out_ap=acc_e[:],
                        in_ap=b16[:, s0 // P:(s0 + sn) // P, :],
                        idxs_ap=cs[:, off + s0 // 16:off + (s0 + sn) // 16],
                        num_idxs=sn,
                        num_idxs_reg=sn,
                        elem_size=D,
                        queue_num=0,
                        sbuf_tokens_per_rank=P,
                        parity_reg=0,
                        out_ap_other=acc_o[:],
                    )
                off += g

            # ---- ctx reduce: t = CTX*tile + c ; even c in acc_e at
            # g = (CTX//2)*tile + c//2, odd c in acc_o likewise ----
            H = CTX // 2  # 5
            ve = acc_e[:, :T * H, :].rearrange("p (t h) d -> p t h d", h=H)
            vo = acc_o[:, :T * H, :].rearrange("p (t h) d -> p t h d", h=H)
            t2 = tpool.tile([P, T, 2, D], bf16, tag="t2")
            nc.vector.tensor_tensor(
                out=t2[:], in0=ve[:, :, 0:2, :], in1=ve[:, :, 2:4, :], op=AL.add)
            u2 = tpool.tile([P, T, 2, D], bf16, tag="u2")
            nc.vector.tensor_tensor(
                out=u2[:], in0=vo[:, :, 0:2, :], in1=vo[:, :, 2:4, :], op=AL.add)
            t1 = tpool.tile([P, T, 1, D], bf16, tag="t1")
            nc.vector.tensor_tensor(
                out=t1[:], in0=t2[:, :, 0:1, :], in1=t2[:, :, 1:2, :], op=AL.add)
            u1 = tpool.tile([P, T, 1, D], bf16, tag="u1")
            nc.vector.tensor_tensor(
                out=u1[:], in0=u2[:, :, 0:1, :], in1=u2[:, :, 1:2, :], op=AL.add)
            t0 = tpool.tile([P, T, 1, D], bf16, tag="t0")
            nc.vector.tensor_tensor(
                out=t0[:], in0=t1[:], in1=ve[:, :, 4:5, :], op=AL.add)
            u0 = tpool.tile([P, T, 1, D], bf16, tag="u0")
            nc.vector.tensor_tensor(
                out=u0[:], in0=u1[:], in1=vo[:, :, 4:5, :], op=AL.add)
            united = cpool.tile([P, T, D], bf16)
            nc.vector.tensor_tensor(
                out=united[:], in0=t0[:, :, 0, :], in1=u0[:, :, 0, :], op=AL.add)
            negated = cpool.tile([P, T, D], bf16)
            nc.vector.tensor_scalar(
                out=negated[:], in0=united[:], scalar1=-1.0, scalar2=None,
                op0=AL.mult)

            # ---- stage +-ctx_sum and the zero row ----
            pos_view = staging[0:plan.items, :].rearrange(
                "(t p) d -> p t d", p=P)
            neg_view = staging[plan.items:2 * plan.items, :].rearrange(
                "(t p) d -> p t d", p=P)
            nc.sync.dma_start(out=pos_view, in_=united[:])
            nc.sync.dma_start(out=neg_view, in_=negated[:])
            zrow = cpool.tile([P, 1, D], bf16)
            nc.vector.memset(zrow[:], 0.0)
            z_view = staging[plan.z_row:plan.z_row + P, :].rearrange(
                "(a p) d -> p a d", p=P)
            nc.sync.dma_start(out=z_view, in_=zrow[:])

            # ---- out phase ----
            off = 0
            col = 0
            for oi, (w, n) in enumerate(plan.out_calls):
                g = n // 16
                gr = n // P
                raw = gpool.tile([P, CHUNK // P, D], f32, tag="raw")
                nc.gpsimd.dma_gather(
                    out_ap=raw[:, :gr, :],
                    in_ap=w_out[win_starts[w]:win_ends[w], :],
                    idxs_ap=og[:, off:off + g],
                    num_idxs=n,
                    num_idxs_reg=n,
                    elem_size=D,
                    queue_num=0,
                )
                ob = gpool.tile([P, CHUNK // P, D], bf16, tag="b16")
                nc.scalar.activation(
                    out=ob[:, :gr, :], in_=raw[:, :gr, :], func=AF.Copy)
                pair = gpool.tile([P, CHUNK // P, D], bf16, tag="pair")
                nc.gpsimd.dma_gather(
                    out_ap=pair[:, :gr, :],
                    in_ap=staging[:][:, :],
                    idxs_ap=op[:, off:off + g],
                    num_idxs=n,
                    num_idxs_reg=n,
                    elem_size=D,
                    queue_num=0,
                )
                nc.vector.tensor_tensor(
                    out=ob[:, :gr, :], in0=ob[:, :gr, :], in1=pair[:, :gr, :],
                    op=AL.mult)
                nc.vector.tensor_reduce(
                    out=score_buf[:, col:col + gr],
                    in_=ob[:, :gr, :],
                    axis=mybir.AxisListType.X,
                    op=AL.add)
                off += g
                col += gr

            # ---- softplus + accumulate ----
            es = cpool.tile([P, plan.score_cols], f32)
            nc.scalar.activation(
                out=es[:], in_=score_buf[:], func=AF.Exp, scale=1.0 / CTX)
            ls = cpool.tile([P, plan.score_cols], f32)
            acc = cpool.tile([P, 1], f32)
            nc.scalar.activation(
                out=ls[:], in_=es[:], func=AF.Ln, bias=1.0, accum_out=acc[:])
            nc.sync.dma_start(out=out[:][:, :], in_=acc[:])

    if not nc.is_finalized():
        nc.finalize()
    return nc


def _get_program(T):
    if T not in _PROGRAM_CACHE:
        _PROGRAM_CACHE[T] = _build_program(_Plan(T))
    return _PROGRAM_CACHE[T]


def _prep_inputs(W_embed, W_out, context_ids, target_ids, neg_ids):
    B = context_ids.shape[0]
    assert B % (NCORES * P) == 0, B
    T = B // (NCORES * P)
    plan = _Plan(T)

    w_e = np.ascontiguousarray(np.asarray(W_embed, dtype=np.float32))
    w_o = np.ascontiguousarray(np.asarray(W_out, dtype=np.float32))
    ctx = np.asarray(context_ids).reshape(NCORES, plan.items, CTX)
    tgt = np.asarray(target_ids).reshape(NCORES, plan.items)
    neg = np.asarray(neg_ids).reshape(NCORES, plan.items, NOUT - 1)

    in_maps = []
    for c in range(NCORES):
        m = _host_prep_core(plan, ctx[c], tgt[c], neg[c])
        m["w_embed"] = w_e
        m["w_out"] = w_o
        in_maps.append(m)
    return in_maps, B, T, plan


def _run(W_embed, W_out, context_ids, target_ids, neg_ids, **spmd_kwargs):
    from concourse import bass_utils

    in_maps, B, T, plan = _prep_inputs(
        W_embed, W_out, context_ids, target_ids, neg_ids)
    nc = _get_program(T)
    res = bass_utils.run_bass_kernel_spmd(
        nc, in_maps, core_ids=list(range(NCORES)), **spmd_kwargs)
    total = 0.0
    for r in res.results:
        total += float(r["out"].astype(np.float64).sum())
    total -= NCORES * plan.out_pads * LN2
    loss = np.float32(total / B)
    return loss, res


def kernel(W_embed, W_out, context_ids, target_ids, neg_ids):
    loss, _ = _run(W_embed, W_out, context_ids, target_ids, neg_ids)
    return loss
